# revision 58
# baseline (speedup 1.0000x reference)
"""Trainium2 Bass kernel v17 (final) for nn_AttLayer (4-head attention, softmax over
queries). Head-lag pipeline with fp8 DoubleRow for the M2 (value)
contraction, tuned for the cold (HAM-throttled, 1.2 GHz PE) regime:

  - es and xtr tiles are float8e4 (e4m3); exp gets bias=-ln(64) so
    es' = es/64 and den' = den/64, which makes xtr' = 64*xtr automatically
    -- both land in e4m3's normal range and M2 = sum xtr' (x) es' is exactly
    the unscaled value (measured rel err 2.9e-3 vs the 2e-2 gate)
  - M2 matmuls run in MatmulPerfMode.DoubleRow: lhsT = xtr[:, 2t:2t+2, :]
    ([128, 2, 65], pair stride padded to 80 B for the dual-fp8 LDWEIGHTS
    16 B alignment rule) and rhs = es[:, 2t:2t+2, ic*512:] ([128, 2, 512])
    contract two j-tiles per instruction -- halves M2 PE time
  - R for heads 1-3 precomputed during head 0's jt loop (PE filler where the
    young pipeline would stall); prev-head M2 pairs injected at jts 0-3 with
    conv at jt 4 and out2 at jt 5, emitted BEFORE each jt's scores so ready
    work never queues behind a ring-stalled scores matmul on the in-order PE
  - all dens via ACT accum_out (fast READ_ACCUMULATOR beats a DVE reduce on
    the tail critical path); 5 warmup matmuls on garbage SBUF fill the
    input-DMA wait and give the HAM clock gate a chance to release early
  - last head's j-tile 7 skips the M2 psum entirely (W-path):
    w7 = rec7*(xa_jt7^T @ ffm) via a cheap 64-col matmul, then
    out2 += w7^T @ es7 (bf16 stationary x fp8 moving) -- the M2 psum closes
    at j-tile 6 so its convs overlap the final exp
  - single full-qa DMA on the scalar HWDGE queue (fast queue, one
    descriptor-gen); xt on the gpsimd SWDGE queue; bf16 output; tail
    conv(ACT)/out2(PE)/add(DVE) stages pipeline 512-wide with the out-DMA
    descriptor-gens issued after the adds so they never stall the conv
    stream; h2+h3 out2 share one accumulating psum
"""

import numpy as np
import ml_dtypes

import concourse.tile as tile
from concourse import bacc, mybir
from concourse.bass_utils import run_bass_kernel_spmd

NH = 4
D = 640
C = 64
CA = C + 1
SEQ = 1024
SCALE = float(D) ** -0.5
N_CORES = 8
FP = mybir.dt.float32
BF = mybir.dt.bfloat16

JT = SEQ // 128
IC = SEQ // 512

AF = mybir.ActivationFunctionType
ALU = mybir.AluOpType
F8 = mybir.dt.float8e4
DR = mybir.MatmulPerfMode.DoubleRow
LN_ALPHA = float(np.log(64.0))
# dual-fp8 LDWEIGHTS requires the pair-dim byte stride to be 16B-aligned
CA_PAD = 80

DVE_DEN_JTS = ()


def _build():
    nc = bacc.Bacc(None, target_bir_lowering=False)
    W_QA = SEQ + NH * CA + NH * C
    qa = nc.declare_dram_parameter("qa", [CA, W_QA], BF, isOutput=False)
    xt = nc.declare_dram_parameter("xt", [128, JT * CA], BF, isOutput=False)
    out = nc.declare_dram_parameter("out", [C, SEQ], BF, isOutput=True)

    with tile.TileContext(nc) as tc:
        with (
            tc.tile_pool(name="consts", bufs=1) as consts,
            tc.tile_pool(name="hpool", bufs=4) as hpool,
            tc.tile_pool(name="sc", bufs=2, space="PSUM") as sc_psum,
            tc.tile_pool(name="pm", bufs=2, space="PSUM") as pm_psum,
        ):
            qa_sb = consts.tile([CA, W_QA], BF)
            nc.scalar.dma_start(out=qa_sb[:], in_=qa[:, :])
            xtb_sb = consts.tile([128, JT * CA], BF)
            xa_sb = qa_sb[:, 0:SEQ]

            def gt_view(h):
                return qa_sb[:, SEQ + h * CA: SEQ + (h + 1) * CA]

            def ff_view(h):
                return qa_sb[:, SEQ + NH * CA + h * C: SEQ + NH * CA + (h + 1) * C]

            def xt_view(jt):
                return xtb_sb[:, jt * CA:(jt + 1) * CA]

            out_sb = consts.tile([C, SEQ], BF)
            o2acc = consts.tile([C, SEQ], FP)
            biasc = consts.tile([128, 1], FP)
            nc.vector.memset(biasc[:], -LN_ALPHA)

            # PE warmup: dummy matmuls on garbage SBUF (no input deps) fill
            # the input-DMA wait so the HAM clock gate releases (K=4/8 ->
            # 8/8, 1.2 -> 2.4 GHz) before real work arrives. Results land in
            # a scratch PSUM tile and are never read.
            warm_w = consts.tile([128, 512], BF)
            nc.vector.memset(warm_w[:], 1.0)
            warm_p = sc_psum.tile([128, SEQ], FP, tag="sc", name="warm")
            for _ in range(5):
                nc.tensor.matmul(
                    warm_p[:, 0:512],
                    lhsT=warm_w[:, 0:128],
                    rhs=warm_w[:, 0:512],
                    start=True, stop=True,
                )

            def emit_late_dmas():
                nc.gpsimd.dma_start(out=xtb_sb[:], in_=xt[:, :])

            def emit_R_ic(h, ic, state, copy=True):
                if ic == 0:
                    state = (
                        hpool.tile([CA, SEQ], BF, tag="R", name=f"R_{h}"),
                        pm_psum.tile([CA, SEQ], FP, tag="pm", name=f"rp_{h}"),
                    )
                R_sb, rps = state
                nc.tensor.matmul(
                    rps[:, ic * 512:(ic + 1) * 512],
                    lhsT=gt_view(h),
                    rhs=xa_sb[:, ic * 512:(ic + 1) * 512],
                    start=True, stop=True,
                )
                if copy:
                    nc.vector.tensor_copy(
                        out=R_sb[:, ic * 512:(ic + 1) * 512],
                        in_=rps[:, ic * 512:(ic + 1) * 512],
                    )
                return state

            def emit_R(h):
                state = emit_R_ic(h, 0, None)
                state = emit_R_ic(h, 1, state)
                return state[0]

            def emit_M2_mms(mps, xtr, es, t, stop_t=JT // 2 - 1):
                # DoubleRow: contract j-tiles 2t and 2t+1 in one instruction
                for ic in range(IC):
                    nc.tensor.matmul(
                        mps[:, ic * 512:(ic + 1) * 512],
                        lhsT=xtr[:, 2 * t:2 * t + 2, 0:CA],
                        rhs=es[:, 2 * t:2 * t + 2, ic * 512:(ic + 1) * 512],
                        start=(t == 0), stop=(t == stop_t),
                        perf_mode=DR,
                    )

            def emit_m2_conv(ph, pmps):
                pm2 = hpool.tile([CA, SEQ], BF, tag="m2", name=f"m2_{ph}")
                for ic in range(IC):
                    nc.vector.tensor_copy(
                        out=pm2[:, ic * 512:(ic + 1) * 512],
                        in_=pmps[:, ic * 512:(ic + 1) * 512],
                    )
                return pm2

            def emit_out2(h, m2, o2p=None, start=True, stop=True):
                if o2p is None:
                    o2p = pm_psum.tile([CA, SEQ], FP, tag="pm", name=f"o2_{h}")
                for ic in range(IC):
                    nc.tensor.matmul(
                        o2p[:C, ic * 512:(ic + 1) * 512],
                        lhsT=ff_view(h),
                        rhs=m2[:, ic * 512:(ic + 1) * 512],
                        start=start, stop=stop,
                    )
                if h == 1:
                    # h0+h1 accumulated in one psum: single copy, no add
                    nc.vector.tensor_copy(out=o2acc[:], in_=o2p[:C, :])
                return o2p

            o2tail = [None]
            o2share = [None]
            w7ref = []

            R_cur = emit_R(0)
            emit_late_dmas()
            R_all = [R_cur, None, None, None]
            R_state = [None]
            prev = None   # (h, es, xtr, mps) of the previous head
            for h in range(NH):
                R_sb = R_all[h]
                last = h == NH - 1
                es = hpool.tile([128, JT, SEQ], F8, tag="es", name=f"es_{h}")
                xtr = hpool.tile([128, JT, CA_PAD], F8, tag="xtr", name=f"xtr_{h}")
                den = hpool.tile([128, JT], FP, tag="den", name=f"den_{h}")
                rec = hpool.tile([128, JT], FP, tag="rec", name=f"rec_{h}")
                own_mps = (
                    pm_psum.tile([CA, SEQ], FP, tag="pm", name="mp_last")
                    if last else None
                )

                for jt in range(JT):
                    # ---- ready injection work FIRST: on the in-order PE
                    # queue it must not sit behind a ring-stalled scores mm.
                    # Previous head's M2 pairs at jts 0-3, conv at jt 4,
                    # out2 at jt 5 (pair 0 fills the head-boundary stall)
                    if prev is not None:
                        ph, pes, pxtr, pmps, pstash = prev
                        if jt <= 3:
                            emit_M2_mms(pmps, pxtr, pes, jt)
                        if jt == 4:
                            pstash.append(emit_m2_conv(ph, pmps))
                        if jt == 5:
                            if ph == 0:
                                o2share[0] = emit_out2(
                                    ph, pstash[0], stop=False)
                            elif ph == 1:
                                emit_out2(ph, pstash[0], o2p=o2share[0],
                                          start=False)
                            else:  # ph == NH - 2
                                o2tail[0] = emit_out2(ph, pstash[0],
                                                      stop=False)
                            prev = None
                    # R for heads 1-3 during head 0's jt loop: real PE
                    # filler that also frees later heads' slots
                    if h == 0 and jt < 2 * (NH - 1):
                        rh = jt // 2 + 1
                        if jt % 2 == 0:
                            R_state[0] = emit_R_ic(rh, 0, None, copy=False)
                        else:
                            st = emit_R_ic(rh, 1, R_state[0], copy=False)
                            # single full-width psum->sbuf cast: one DVE
                            # dispatch+bubble instead of two
                            nc.vector.tensor_copy(out=st[0][:], in_=st[1][:])
                            R_all[rh] = st[0]

                    pst = sc_psum.tile([128, SEQ], FP, tag="sc", name=f"sc_{h}_{jt}")
                    for ic in range(IC):
                        nc.tensor.matmul(
                            pst[:, ic * 512:(ic + 1) * 512],
                            lhsT=xa_sb[:, jt * 128:(jt + 1) * 128],
                            rhs=R_sb[:, ic * 512:(ic + 1) * 512],
                            start=True, stop=True,
                        )
                    if jt in DVE_DEN_JTS:
                        nc.scalar.activation(
                            out=es[:, jt, :], in_=pst[:],
                            func=AF.Exp, scale=SCALE, bias=biasc[:],
                        )
                        nc.vector.tensor_reduce(
                            out=den[:, jt:jt + 1], in_=es[:, jt, :],
                            axis=mybir.AxisListType.X, op=ALU.add,
                        )
                    else:
                        nc.scalar.activation(
                            out=es[:, jt, :], in_=pst[:],
                            func=AF.Exp, scale=SCALE, bias=biasc[:],
                            accum_out=den[:, jt:jt + 1],
                        )
                    nc.vector.reciprocal(out=rec[:, jt:jt + 1], in_=den[:, jt:jt + 1])
                    if not (last and jt == 7):
                        # jt7 of the last head goes through the W-path; its
                        # xtr is never read
                        nc.vector.tensor_scalar_mul(
                            xtr[:, jt, 0:CA], xt_view(jt), rec[:, jt:jt + 1],
                        )

                    if last and jt >= 2 and jt % 2 == 0:
                        emit_M2_mms(own_mps, xtr, es, jt // 2 - 1, stop_t=-1)
                    if last and jt == 7:
                        # j-tile 6 singles close own_mps (stop=True): the
                        # psum->sbuf convs no longer gate on j-tile 7
                        for ic in range(IC):
                            nc.tensor.matmul(
                                own_mps[:, ic * 512:(ic + 1) * 512],
                                lhsT=xtr[:, 6, 0:CA],
                                rhs=es[:, 6, ic * 512:(ic + 1) * 512],
                                start=False, stop=True,
                            )
                        # W-path for j-tile 7: W7' = xa_jt7^T @ ffm3 (64-col
                        # matmul, inputs ready now); out2 += (rec7*W7')^T es7
                        # later -- no M2 psum pass for jt7 at all
                        w7ref.append(sc_psum.tile(
                            [128, SEQ], FP, tag="sc", name="w7p"))
                        nc.tensor.matmul(
                            w7ref[0][:, 0:C],
                            lhsT=xa_sb[:, 7 * 128:8 * 128],
                            rhs=ff_view(NH - 1),
                            start=True, stop=True,
                        )

                if not last:
                    mps = pm_psum.tile([CA, SEQ], FP, tag="pm", name=f"mp_{h}")
                    prev = (h, es, xtr, mps, [])

            # ---- tail: last head's M2 pair 3 (j-tiles 6,7), conversion,
            # out2, output
            w7_sb = consts.tile([128, C], BF)
            nc.vector.tensor_scalar_mul(
                w7_sb[:], w7ref[0][:, 0:C], rec[:, 7:8],
            )
            pm2 = hpool.tile([CA, SEQ], BF, tag="m2", name="m2_last")
            o2p = o2tail[0]  # h2's out2 psum, kept open with stop=False
            # jt7's out2 via the W-path: needs only w7 + es7
            for ic in range(IC):
                sl = slice(ic * 512, (ic + 1) * 512)
                nc.tensor.matmul(
                    o2p[:C, sl],
                    lhsT=w7_sb[:],
                    rhs=es[:, 7, sl],
                    start=False, stop=False,
                )
            # 512-wide conv(ACT)/out2(PE)/add(DVE) stages pipeline across
            # three engines; dges go last so they never stall the conv stream
            # the two psum->sbuf convs run in parallel on ACT and DVE
            nc.scalar.copy(out=pm2[:, 0:512], in_=own_mps[:, 0:512])
            nc.vector.tensor_copy(out=pm2[:, 512:1024], in_=own_mps[:, 512:1024])
            for k in range(IC):
                sl = slice(k * 512, (k + 1) * 512)
                nc.tensor.matmul(
                    o2p[:C, sl],
                    lhsT=ff_view(NH - 1),
                    rhs=pm2[:, sl],
                    start=False, stop=True,
                )
                nc.vector.tensor_add(
                    out=out_sb[:, sl], in0=o2p[:C, sl], in1=o2acc[:, sl],
                )
            for k in range(IC):
                sl = slice(k * 512, (k + 1) * 512)
                nc.scalar.dma_start(out=out[:, sl], in_=out_sb[:, sl])

    nc.compile()
    return nc


_CACHE: dict = {}


def _get_nc():
    if "nc" not in _CACHE:
        _CACHE["nc"] = _build()
    return _CACHE["nc"]


def _prep_in_maps(x, W_proj, b_proj, W_out, b_out):
    bf = ml_dtypes.bfloat16
    x = np.ascontiguousarray(np.asarray(x, dtype=np.float32))
    W_proj = np.asarray(W_proj, dtype=np.float32)
    b_proj = np.asarray(b_proj, dtype=np.float32)
    W_out = np.asarray(W_out, dtype=np.float32)

    x2 = x.reshape(N_CORES, C, SEQ)
    W_QA = SEQ + NH * CA + NH * C

    Wa = np.concatenate([W_proj, b_proj[None, :]], axis=0)  # [65, 7680]
    gt = np.empty((CA, NH, CA), dtype=np.float32)
    ffm = np.empty((CA, NH, C), dtype=np.float32)
    for h in range(NH):
        q0 = h * 3 * D
        Wq = Wa[:, q0:q0 + D]
        Wk = Wa[:, q0 + D:q0 + 2 * D]
        Wv = Wa[:, q0 + 2 * D:q0 + 3 * D]
        G = Wk @ Wq.T
        gt[:, h, :] = G.T
        ffm[:, h, :] = Wv @ W_out[h * D:(h + 1) * D, :]

    qa_all = np.empty((N_CORES, CA, W_QA), dtype=bf)
    qa_all[:, :C, :SEQ] = x2.astype(bf)
    qa_all[:, C, :SEQ] = np.float32(1.0)
    qa_all[:, :, SEQ:SEQ + NH * CA] = gt.reshape(CA, NH * CA).astype(bf)[None]
    qa_all[:, :, SEQ + NH * CA:] = ffm.reshape(CA, NH * C).astype(bf)[None]

    xt_all = np.empty((N_CORES, 128, JT, CA), dtype=bf)
    xtt = x2.transpose(0, 2, 1).reshape(N_CORES, JT, 128, C)
    xt_all[:, :, :, :C] = xtt.transpose(0, 2, 1, 3).astype(bf)
    xt_all[:, :, :, C] = np.float32(1.0)
    xt_all = xt_all.reshape(N_CORES, 128, JT * CA)

    return [
        {
            "qa": np.ascontiguousarray(qa_all[i]),
            "xt": np.ascontiguousarray(xt_all[i]),
        }
        for i in range(N_CORES)
    ]


def run(x, t, W_proj, b_proj, W_out, b_out, trace=False, **trace_kwargs):
    x = np.ascontiguousarray(np.asarray(x, dtype=np.float32))
    in_maps = _prep_in_maps(x, W_proj, b_proj, W_out, b_out)
    res = run_bass_kernel_spmd(
        _get_nc(), in_maps, core_ids=list(range(N_CORES)),
        trace=trace, **trace_kwargs,
    )
    out2 = np.stack([np.asarray(res.results[i]["out"], dtype=np.float32)
                     for i in range(N_CORES)])
    b_out = np.asarray(b_out, dtype=np.float32)
    full = out2 + x.reshape(N_CORES, C, SEQ) + b_out[None, :, None]
    return full.reshape(N_CORES, C, 32, 32), res


def kernel(x, t=None, W_proj=None, b_proj=None, W_out=None, b_out=None):
    out, _ = run(x, t, W_proj, b_proj, W_out, b_out, trace=False)
    return out

